# revision 7
# baseline (speedup 1.0000x reference)
"""Trainium2 Bass kernel for nn_Discriminator (segment_reduce, 8 cores).

Math (collapsed form of the reference):
  The reference projects the full embedding table (emb = E @ W_i.T + b_i),
  gathers pos/neg rows, does a segment-mean over pos rows, and scores each
  row with a bilinear form against its segment embedding.  Everything is
  linear, so it collapses to operations on RAW embedding rows:

    m[s]     = mean of raw E rows of segment s's pos samples        [256]
    grid[s]  = W_i m[s] + b_i
    h[s]     = Wb grid[s]                  (Wb = W_k[0])
    u[s]     = W_i^T h[s];   c[s] = b_i . h[s] + b_k
    logit[n] = E[idx[n]] . u[seg(n)] + c[seg(n)]

  The 1/seg_size mean scaling is folded into a host-prescaled W_i, so the
  device only needs segment SUMS.

Sharding: data-parallel over samples, segments kept whole per core
(core k owns segments [k*128, (k+1)*128)).  Fully local, no collectives.

Device pipeline per core:
  - The host stages each core's sampled rows (pos then neg, in processing
    order) as bf16 in a feature-transposed block layout; the device
    streams them with large sequential HWDGE DMAs at full bus rate
    (~50 MB/core -- the memory roofline for this regime).  Layout:
    rows[p, ci*2048 + c*1024 + n] = feature c*128+p of row n of call ci.
  - Segment sums: one 3-D DVE tensor_reduce per (pos call, chunk)
    reduces the innermost 128 rows for 8 segments at once.
  - The tiny u-chain (256x256 matmuls on 16 segment vectors) runs per
    group of 16 segments; u stays as COLUMNS (feature on partitions), so
    it feeds the dot matmuls directly -- no transpose, no broadcast.
  - Per 128-row block: 2 accumulating PE matmuls (lhsT = transposed rows
    chunk [128x128], rhs = u column [128x1]) produce the block's logits
    as a PSUM column.  8 blocks share a PSUM tile; one DVE copy per call
    moves [128,8] logits to SBUF.
  - Final: c broadcast on PE (ones-column outer product) + DVE add.
"""

import numpy as np
import ml_dtypes

import concourse.bass as bass
import concourse.bacc as bacc
import concourse.mybir as mybir
from concourse import bass_utils
from concourse.tile import TileContext

F32 = mybir.dt.float32
BF16 = mybir.dt.bfloat16

N_NODES = 200000
H = 256
N_SEG = 1024
SEG_SZ = 128          # rows per segment (asserted at runtime)
N_POS = N_SEG * SEG_SZ          # 131072
NEG_RATIO = 5
N_NEG = N_POS * NEG_RATIO       # 655360
N_CORES = 8

SEG_PC = N_SEG // N_CORES       # 128 segments per core
POS_PC = N_POS // N_CORES       # 16384
NEG_PC = N_NEG // N_CORES       # 81920
P = 128
POS_BLK = POS_PC // P           # 128 blocks (block == segment for pos)
NEG_BLK = NEG_PC // P           # 640 blocks (5 consecutive per segment)
TOT_BLK = POS_BLK + NEG_BLK     # 768 logit columns

GB = 8                          # blocks per "call" (1024 rows)
CALL_IDX = GB * P               # 1024 rows per call
CALL_W = 2 * CALL_IDX           # 2048 bf16 columns per call tile
NEG_BUFS = 12                   # in-flight neg tiles
GSEG = 16                       # segments per u-chain group
NGRP = SEG_PC // GSEG           # 8 groups
POS_CALLS = POS_BLK // GB       # 16 (2 per group)
NEG_CALLS = NEG_BLK // GB       # 80 (10 per group)
N_CALLS = POS_CALLS + NEG_CALLS # 96

_CACHED = None


def _build_module() -> bass.Bass:
    # Bacc (not raw Bass): its compile() pass splits multi-sem waits into
    # event semaphores — walrus rejects >1 sync wait per instruction.
    nc = bacc.Bacc("TRN2", target_bir_lowering=False, debug=False)

    rows = nc.dram_tensor("rows", [P, N_CALLS * CALL_W], BF16,
                          kind="ExternalInput")
    # w_iT[p, j, f'] = (W_i/seg_sz).T[j*128+p, f']  (lhsT tiles, G = W_i @ M)
    w_iT = nc.dram_tensor("w_iT", [P, 2, H], F32, kind="ExternalInput")
    # wbT[p, j, d]  = Wb.T[j*128+p, d]        (lhsT tiles for H = Wb @ G)
    wbT = nc.dram_tensor("wbT", [P, 2, H], F32, kind="ExternalInput")
    # w_ext[p, j, m] = [W_i | b_i][j*128+p, m]  (lhsT tiles for U~ = W_ext^T H)
    w_ext = nc.dram_tensor("w_ext", [P, 2, H + 1], F32, kind="ExternalInput")
    b_i2 = nc.dram_tensor("b_i2", [P, 2], F32, kind="ExternalInput")
    b_k = nc.dram_tensor("b_k", [1, 1], F32, kind="ExternalInput")
    logits_d = nc.dram_tensor("logits", [P, TOT_BLK], F32, kind="ExternalOutput")

    W1 = H + 1

    with TileContext(nc) as tc:
        with (
            tc.tile_pool(name="const", bufs=1) as const,
            tc.tile_pool(name="grp", bufs=2) as grp,
            tc.tile_pool(name="ucols", bufs=3) as ucolsp,
            tc.tile_pool(name="pospool", bufs=NGRP) as pospool,
            tc.tile_pool(name="negpool", bufs=NEG_BUFS) as negpool,
            tc.tile_pool(name="chain", bufs=4, space="PSUM") as chainp,
            tc.tile_pool(name="dot", bufs=3, space="PSUM") as dotp,
            tc.tile_pool(name="cb", bufs=1, space="PSUM") as cbp,
        ):
            # ---- tiles ----
            ones1 = const.tile([1, P], F32, tag="ones1")
            nc.gpsimd.memset(ones1[:], 1.0)

            w_iT_sb = const.tile([P, 2 * H], F32, tag="wiT")
            wbT_sb = const.tile([P, 2 * H], F32, tag="wbT")
            w_ext_sb = const.tile([P, 2 * W1], F32, tag="wext")
            b_i2_sb = const.tile([P, 2], F32, tag="bi2")
            b_k_sb = const.tile([1, 1], F32, tag="bk")

            logits_sb = const.tile([P, TOT_BLK], F32, tag="logits")
            c_row = const.tile([1, TOT_BLK], F32, tag="crow")

            # ---- streaming loads ----
            # pos: one DMA per group (2 calls = 1 MB); neg: one per call.
            pos_gtiles = [None] * NGRP
            neg_tiles = [None] * NEG_CALLS

            def emit_pos_group(g):
                pt = pospool.tile([P, 2 * CALL_W], BF16, tag="pos")
                pos_gtiles[g] = pt
                nc.sync.dma_start(
                    pt[:], rows[:, 2 * g * CALL_W:(2 * g + 2) * CALL_W])

            def emit_neg(gi):
                t = negpool.tile([P, CALL_W], BF16, tag="neg")
                neg_tiles[gi] = t
                nc.sync.dma_start(
                    t[:], rows[:, (POS_CALLS + gi) * CALL_W:
                               (POS_CALLS + gi + 1) * CALL_W])

            NEG_PER_GRP = NEG_CALLS // NGRP             # 10 neg calls/group
            emit_pos_group(0)
            emit_pos_group(1)
            # weights ride behind the first two pos streams (they are only
            # needed once the first group's sums are done)
            nc.sync.dma_start(w_iT_sb[:], w_iT[:, :, :])
            nc.sync.dma_start(wbT_sb[:], wbT[:, :, :])
            nc.sync.dma_start(w_ext_sb[:], w_ext[:, :, :])
            nc.sync.dma_start(b_i2_sb[:], b_i2[:, :])
            nc.sync.dma_start(b_k_sb[:], b_k[:, :])
            for g in range(NGRP):
                for i in range(NEG_PER_GRP):
                    emit_neg(g * NEG_PER_GRP + i)
                    if i == 5 and g + 2 < NGRP:
                        emit_pos_group(g + 2)

            # ---- per group of GSEG segments: sums + u-chain + dots ----
            for g in range(NGRP):
                s0 = g * GSEG
                pgt = pos_gtiles[g]

                # segment sums straight from the transposed tiles:
                # mT[p, c*16 + sloc] = sum over rows of feature c*128+p
                mT = grp.tile([P, 2 * GSEG], F32, tag="mT")
                for cal in range(2):
                    for c in range(2):
                        nc.vector.tensor_reduce(
                            out=mT[:, c * GSEG + cal * GB:
                                   c * GSEG + cal * GB + GB],
                            in_=pgt[:, cal * CALL_W + c * CALL_IDX:
                                    cal * CALL_W + (c + 1) * CALL_IDX]
                                .rearrange("p (s n) -> p s n", s=GB),
                            op=mybir.AluOpType.add,
                            axis=mybir.AxisListType.X,
                        )

                # G_T = (W_i/seg_sz) @ M_T + b_i
                gT = grp.tile([P, 2 * GSEG], F32, tag="gT")
                for t in range(2):
                    pg = chainp.tile([P, GSEG], F32, tag="chain")
                    for j in range(2):
                        nc.tensor.matmul(
                            out=pg[:],
                            lhsT=w_iT_sb[:, j * H + t * P: j * H + t * P + P],
                            rhs=mT[:, j * GSEG:(j + 1) * GSEG],
                            start=(j == 0),
                            stop=(j == 1),
                        )
                    nc.vector.tensor_scalar(
                        out=gT[:, t * GSEG:(t + 1) * GSEG], in0=pg[:],
                        scalar1=b_i2_sb[:, t:t + 1], scalar2=None,
                        op0=mybir.AluOpType.add,
                    )

                # H_T = Wb @ G_T
                hT = grp.tile([P, 2 * GSEG], F32, tag="hT")
                for t in range(2):
                    ph = chainp.tile([P, GSEG], F32, tag="chain")
                    for j in range(2):
                        nc.tensor.matmul(
                            out=ph[:],
                            lhsT=wbT_sb[:, j * H + t * P: j * H + t * P + P],
                            rhs=gT[:, j * GSEG:(j + 1) * GSEG],
                            start=(j == 0),
                            stop=(j == 1),
                        )
                    nc.vector.tensor_copy(hT[:, t * GSEG:(t + 1) * GSEG], ph[:])

                # U~_T = [W_i | b_i]^T @ H_T, kept as bf16 COLUMNS
                u_cols = ucolsp.tile([P, 2 * GSEG], BF16, tag="ucols")
                for t in range(2):
                    pu = chainp.tile([P, GSEG], F32, tag="chain")
                    for j in range(2):
                        nc.tensor.matmul(
                            out=pu[:],
                            lhsT=w_ext_sb[:, j * W1 + t * P: j * W1 + t * P + P],
                            rhs=hT[:, j * GSEG:(j + 1) * GSEG],
                            start=(j == 0),
                            stop=(j == 1),
                        )
                    nc.vector.tensor_copy(u_cols[:, t * GSEG:(t + 1) * GSEG],
                                          pu[:])
                # c row: b_i . h + b_k
                puc = chainp.tile([1, GSEG], F32, tag="chain")
                for j in range(2):
                    nc.tensor.matmul(
                        out=puc[:],
                        lhsT=w_ext_sb[:, j * W1 + H: j * W1 + H + 1],
                        rhs=hT[:, j * GSEG:(j + 1) * GSEG],
                        start=(j == 0),
                        stop=(j == 1),
                    )
                uc_sb = grp.tile([1, GSEG], F32, tag="ucsb")
                nc.vector.tensor_scalar(
                    out=uc_sb[:], in0=puc[:], scalar1=b_k_sb[:1, :1],
                    scalar2=None, op0=mybir.AluOpType.add,
                )

                # c values for this group (pos cols + 5x-repeated neg cols)
                nc.vector.tensor_copy(c_row[:1, s0:s0 + GSEG], uc_sb[:1, :])
                for r in range(5):
                    nc.vector.tensor_copy(
                        c_row[:1, POS_BLK + 5 * s0 + r:
                              POS_BLK + 5 * s0 + r + 76:5],
                        uc_sb[:1, :])

                # broadcast this group's c values to all partitions once
                # (PE ones-column outer product), so each call's PSUM ->
                # SBUF move is a fused add of the c bias.
                GW = GSEG + NEG_PER_GRP * GB            # 96 columns
                pcb = cbp.tile([P, GW], F32, tag="cb")
                nc.tensor.matmul(
                    out=pcb[:, :GSEG], lhsT=ones1[:],
                    rhs=c_row[:1, s0:s0 + GSEG], start=True, stop=True)
                nc.tensor.matmul(
                    out=pcb[:, GSEG:], lhsT=ones1[:],
                    rhs=c_row[:1, POS_BLK + 5 * s0:POS_BLK + 5 * s0 + 80],
                    start=True, stop=True)
                cb_sb = grp.tile([P, GW], F32, tag="cbsb")
                nc.vector.tensor_copy(cb_sb[:], pcb[:])

                # dots: per call, 8 blocks x 2 accumulating matmuls into a
                # PSUM tile, then one [128,8] add-with-c to the logits tile.
                def emit_dots(tile, coff, blocks, cols0, cb0):
                    pd = dotp.tile([P, GB], F32, tag="dot")
                    for b, sloc in enumerate(blocks):
                        for c in range(2):
                            nc.tensor.matmul(
                                out=pd[:, b:b + 1],
                                lhsT=tile[:, coff + c * CALL_IDX + b * P:
                                          coff + c * CALL_IDX + (b + 1) * P],
                                rhs=u_cols[:, c * GSEG + sloc:
                                           c * GSEG + sloc + 1],
                                start=(c == 0),
                                stop=(c == 1),
                            )
                    nc.vector.tensor_tensor(
                        out=logits_sb[:, cols0:cols0 + GB], in0=pd[:],
                        in1=cb_sb[:, cb0:cb0 + GB], op=mybir.AluOpType.add)

                for cal in range(2):
                    emit_dots(pgt, cal * CALL_W,
                              [cal * GB + b for b in range(GB)],
                              s0 + cal * GB, cal * GB)
                for i in range(NEG_PER_GRP):
                    gi = g * NEG_PER_GRP + i
                    q0 = gi * GB
                    emit_dots(neg_tiles[gi], 0,
                              [(q0 + b) // NEG_RATIO - s0 for b in range(GB)],
                              POS_BLK + q0, GSEG + i * GB)

            nc.sync.dma_start(logits_d[:, :], logits_sb[:])

    nc.compile()
    return nc


def get_module() -> bass.Bass:
    global _CACHED
    if _CACHED is None:
        _CACHED = _build_module()
    return _CACHED


def make_in_maps(inputs: dict) -> list[dict]:
    emb = np.ascontiguousarray(np.asarray(inputs["embedding"], dtype=np.float32))
    gs = np.asarray(inputs["grid_sizes"]).astype(np.int64)
    pos_s = np.asarray(inputs["pos_samples"]).astype(np.int64)
    neg_s = np.asarray(inputs["neg_samples"]).astype(np.int64)
    W_i = np.asarray(inputs["W_i"], dtype=np.float32)
    b_i = np.asarray(inputs["b_i"], dtype=np.float32)
    Wb = np.asarray(inputs["W_k"], dtype=np.float32)[0]
    b_kv = np.asarray(inputs["b_k"], dtype=np.float32)

    if not (gs.shape == (N_SEG,) and np.all(gs == SEG_SZ)):
        raise RuntimeError("kernel assumes grid_sizes == 128 everywhere")
    assert pos_s.shape == (N_POS,) and neg_s.shape == (N_NEG,)

    emb_bf = emb.astype(ml_dtypes.bfloat16)

    # mean = sum/seg_sz folded into the first chain matmul's weights
    w_iT_np = np.ascontiguousarray(
        (W_i / float(SEG_SZ)).T.reshape(2, P, H).transpose(1, 0, 2))
    wbT_np = np.ascontiguousarray(
        Wb.T.reshape(2, P, H).transpose(1, 0, 2))
    W_ext = np.concatenate([W_i, b_i[:, None]], axis=1)        # [256, 257]
    w_ext_np = np.ascontiguousarray(
        W_ext.reshape(2, P, H + 1).transpose(1, 0, 2))
    b_i2_np = np.ascontiguousarray(b_i.reshape(2, P).T)
    b_k_np = b_kv.reshape(1, 1)

    in_maps = []
    for k in range(N_CORES):
        # processing order: pos rows then neg rows of this core, staged in
        # the device's transposed block layout:
        # rows[p, ci*2048 + c*1024 + n] = emb[full[ci*1024+n], c*128+p]
        full = np.concatenate([
            pos_s[k * POS_PC:(k + 1) * POS_PC],
            neg_s[k * NEG_PC:(k + 1) * NEG_PC],
        ])
        g = emb_bf[full]                       # [98304, 256]
        rows_np = np.ascontiguousarray(
            g.reshape(N_CALLS, CALL_IDX, 2, P).transpose(3, 0, 2, 1)
            .reshape(P, N_CALLS * CALL_W))
        in_maps.append({
            "rows": rows_np,
            "w_iT": w_iT_np,
            "wbT": wbT_np,
            "w_ext": w_ext_np,
            "b_i2": b_i2_np,
            "b_k": b_k_np,
        })
    return in_maps


def assemble_output(core_outs: list[np.ndarray]) -> np.ndarray:
    pos_parts, neg_parts = [], []
    for k in range(N_CORES):
        o = np.asarray(core_outs[k])
        assert o.shape == (P, TOT_BLK)
        pos_parts.append(o[:, :POS_BLK].T.ravel())
        neg_parts.append(o[:, POS_BLK:].T.ravel())
    return np.concatenate(pos_parts + neg_parts).astype(np.float32)


def kernel(**inputs) -> np.ndarray:
    nc = get_module()
    in_maps = make_in_maps(inputs)
    res = bass_utils.run_bass_kernel_spmd(
        nc, in_maps, core_ids=list(range(N_CORES)))
    return assemble_output([r["logits"] for r in res.results])


# revision 13
# speedup vs baseline: 2.4638x; 2.4638x over previous
"""Trainium2 Bass kernel for nn_Discriminator (segment_reduce, 8 cores).

Math (collapsed form of the reference):
  The reference projects the full embedding table (emb = E @ W_i.T + b_i),
  gathers pos/neg rows, does a segment-mean over pos rows, and scores each
  row with a bilinear form against its segment embedding.  Everything is
  linear, so it collapses to operations on RAW embedding rows:

    m[s]     = mean of raw E rows of segment s's pos samples        [256]
    grid[s]  = W_i m[s] + b_i
    h[s]     = Wb grid[s]                  (Wb = W_k[0])
    u[s]     = W_i^T h[s];   c[s] = b_i . h[s] + b_k
    logit[n] = E[idx[n]] . u[seg(n)] + c[seg(n)]

  The 1/seg_size mean scaling is folded into a host-prescaled W_i; the
  b_i / b_k / c biases are folded into PE matmuls (ones-row outer
  products), so the chain needs no per-element bias ops at all.

Sharding: data-parallel over samples, segments kept whole per core
(core k owns segments [k*128, (k+1)*128)).  Fully local, no collectives.

Device pipeline per core:
  - The host stages each core's sampled rows (pos then neg, in processing
    order) as bf16 in a feature-transposed block layout; the device
    streams them with large sequential DMAs split across THREE issuing
    engines (SP / Activation HWDGE + gpsimd SWDGE) so the transfers
    pipeline three-wide (~50 MB/core of traffic).
    Layout: rows[p, ci*2048 + c*1024 + n] = feature c*128+p of row n.
  - Segment sums: one 3-D DVE tensor_reduce per (pos call, chunk)
    reduces the innermost 128 rows for 8 segments at once.
  - The tiny u-chain runs per group of 16 segments, software-pipelined
    one group ahead; u stays as COLUMNS so it feeds the dots directly.
  - Per 128-row block: 2 accumulating PE matmuls (lhsT = transposed rows
    chunk [128x128], rhs = u column [128x1]) produce the block's logits
    in a per-group [128,96] PSUM tile whose columns were pre-seeded with
    the c bias by 6 ones-row matmuls; one DVE copy per group moves the
    finished logits to SBUF.  Neg columns are (r,s)-reordered inside the
    group so every c seed is a contiguous 16-column matmul; the host
    unpermutes when assembling the output.
"""

import numpy as np
import ml_dtypes

import concourse.bass as bass
import concourse.bacc as bacc
import concourse.mybir as mybir
from concourse import bass_utils
from concourse.tile import TileContext

F32 = mybir.dt.float32
BF16 = mybir.dt.bfloat16

N_NODES = 200000
H = 256
N_SEG = 1024
SEG_SZ = 128          # rows per segment (asserted at runtime)
N_POS = N_SEG * SEG_SZ          # 131072
NEG_RATIO = 5
N_NEG = N_POS * NEG_RATIO       # 655360
N_CORES = 8

SEG_PC = N_SEG // N_CORES       # 128 segments per core
POS_PC = N_POS // N_CORES       # 16384
NEG_PC = N_NEG // N_CORES       # 81920
P = 128
POS_BLK = POS_PC // P           # 128 blocks (block == segment for pos)
NEG_BLK = NEG_PC // P           # 640 blocks (5 consecutive per segment)
TOT_BLK = POS_BLK + NEG_BLK     # 768 logit columns

GB = 8                          # blocks per "call" (1024 rows)
CALL_IDX = GB * P               # 1024 rows per call
CALL_W = 2 * CALL_IDX           # 2048 bf16 columns per call tile
NEG_BUFS = 12                   # in-flight neg tiles
GSEG = 16                       # segments per u-chain group
NGRP = SEG_PC // GSEG           # 8 groups
POS_CALLS = POS_BLK // GB       # 16 (2 per group)
NEG_CALLS = NEG_BLK // GB       # 80 (10 per group)
N_CALLS = POS_CALLS + NEG_CALLS # 96
NEG_PER_GRP = NEG_CALLS // NGRP # 10
GRP_W = GSEG + NEG_PER_GRP * GB # 96 logit columns per group

# neg-load lane pattern per group (ACT also carries const loads + has the
# least fixed work, so it takes 4 of each 10)
NEG_LANES = ["scalar", "sync", "gpsimd", "scalar", "sync",
             "gpsimd", "scalar", "sync", "gpsimd", "scalar"]

_CACHED = None


def _build_module() -> bass.Bass:
    # Bacc (not raw Bass): its compile() pass splits multi-sem waits into
    # event semaphores — walrus rejects >1 sync wait per instruction.
    nc = bacc.Bacc("TRN2", target_bir_lowering=False, debug=False)

    rows = nc.dram_tensor("rows", [P, N_CALLS * CALL_W], BF16,
                          kind="ExternalInput")
    # w_iT[p, j, f'] = (W_i/seg_sz).T[j*128+p, f']  (lhsT tiles, G = W_i @ M)
    w_iT = nc.dram_tensor("w_iT", [P, 2, H], F32, kind="ExternalInput")
    # wbT[p, j, d]  = Wb.T[j*128+p, d]        (lhsT tiles for H = Wb @ G)
    wbT = nc.dram_tensor("wbT", [P, 2, H], F32, kind="ExternalInput")
    # w_ext[p, j, m] = [W_i | b_i][j*128+p, m]  (lhsT tiles for U~ = W_ext^T H)
    w_ext = nc.dram_tensor("w_ext", [P, 2, H + 1], F32, kind="ExternalInput")
    b_iR = nc.dram_tensor("b_iR", [1, H], F32, kind="ExternalInput")
    b_k = nc.dram_tensor("b_k", [1, 1], F32, kind="ExternalInput")
    logits_d = nc.dram_tensor("logits", [P, TOT_BLK], F32, kind="ExternalOutput")

    W1 = H + 1

    with TileContext(nc) as tc:
        with (
            tc.tile_pool(name="const", bufs=1) as const,
            tc.tile_pool(name="grp", bufs=2) as grp,
            tc.tile_pool(name="ucols", bufs=3) as ucolsp,
            tc.tile_pool(name="pospool", bufs=NGRP) as pospool,
            tc.tile_pool(name="negpool", bufs=NEG_BUFS) as negpool,
            tc.tile_pool(name="chain", bufs=2, space="PSUM") as chainp,
            tc.tile_pool(name="dot", bufs=3, space="PSUM") as dotp,
        ):
            # ---- tiles ----
            ones1 = const.tile([1, P], F32, tag="ones1")
            nc.gpsimd.memset(ones1[:], 1.0)

            w_iT_sb = const.tile([P, 2 * H], F32, tag="wiT")
            wbT_sb = const.tile([P, 2 * H], F32, tag="wbT")
            w_ext_sb = const.tile([P, 2 * W1], F32, tag="wext")
            b_iR_sb = const.tile([1, H], F32, tag="biR")
            b_k_sb = const.tile([1, 1], F32, tag="bk")
            logits_sb = const.tile([P, TOT_BLK], F32, tag="logits")

            pos_gtiles = [None] * NGRP
            neg_tiles = [None] * NEG_CALLS
            u_cols_l = [None] * NGRP
            uc_l = [None] * NGRP
            pd_l = [None] * NGRP

            pos_lane = ["sync", "gpsimd"]

            def emit_pos_group(g):
                pt = pospool.tile([P, 2 * CALL_W], BF16, tag="pos")
                pos_gtiles[g] = pt
                getattr(nc, pos_lane[g % 2]).dma_start(
                    pt[:], rows[:, 2 * g * CALL_W:(2 * g + 2) * CALL_W])

            def emit_neg(gi):
                t = negpool.tile([P, CALL_W], BF16, tag="neg")
                neg_tiles[gi] = t
                getattr(nc, NEG_LANES[gi % NEG_PER_GRP]).dma_start(
                    t[:], rows[:, (POS_CALLS + gi) * CALL_W:
                               (POS_CALLS + gi + 1) * CALL_W])

            def emit_prep(g):
                """Segment sums + u-chain for group g (pos tile must be
                in flight); produces u_cols_l[g] (bf16) and uc_l[g]."""
                pgt = pos_gtiles[g]
                mT = grp.tile([P, 2 * GSEG], F32, tag="mT")
                for cal in range(2):
                    for c in range(2):
                        nc.vector.tensor_reduce(
                            out=mT[:, c * GSEG + cal * GB:
                                   c * GSEG + cal * GB + GB],
                            in_=pgt[:, cal * CALL_W + c * CALL_IDX:
                                    cal * CALL_W + (c + 1) * CALL_IDX]
                                .rearrange("p (s n) -> p s n", s=GB),
                            op=mybir.AluOpType.add,
                            axis=mybir.AxisListType.X,
                        )

                # G_T = (W_i/seg_sz) @ M_T + b_i (bias via ones-row matmul)
                pg = chainp.tile([P, 2 * GSEG], F32, tag="chain")
                for t in range(2):
                    for j in range(2):
                        nc.tensor.matmul(
                            out=pg[:, t * GSEG:(t + 1) * GSEG],
                            lhsT=w_iT_sb[:, j * H + t * P: j * H + t * P + P],
                            rhs=mT[:, j * GSEG:(j + 1) * GSEG],
                            start=(j == 0), stop=False,
                        )
                    nc.tensor.matmul(
                        out=pg[:, t * GSEG:(t + 1) * GSEG],
                        lhsT=b_iR_sb[:1, t * P:(t + 1) * P],
                        rhs=ones1[:1, :GSEG],
                        start=False, stop=True,
                    )
                gT = grp.tile([P, 2 * GSEG], F32, tag="gT")
                nc.vector.tensor_copy(gT[:], pg[:])

                # H_T = Wb @ G_T
                ph = chainp.tile([P, 2 * GSEG], F32, tag="chain")
                for t in range(2):
                    for j in range(2):
                        nc.tensor.matmul(
                            out=ph[:, t * GSEG:(t + 1) * GSEG],
                            lhsT=wbT_sb[:, j * H + t * P: j * H + t * P + P],
                            rhs=gT[:, j * GSEG:(j + 1) * GSEG],
                            start=(j == 0), stop=(j == 1),
                        )
                hT = grp.tile([P, 2 * GSEG], F32, tag="hT")
                nc.scalar.copy(out=hT[:], in_=ph[:])

                # U~_T = [W_i | b_i]^T @ H_T, kept as bf16 COLUMNS
                pu = chainp.tile([P, 2 * GSEG], F32, tag="chain")
                for t in range(2):
                    for j in range(2):
                        nc.tensor.matmul(
                            out=pu[:, t * GSEG:(t + 1) * GSEG],
                            lhsT=w_ext_sb[:, j * W1 + t * P: j * W1 + t * P + P],
                            rhs=hT[:, j * GSEG:(j + 1) * GSEG],
                            start=(j == 0), stop=(j == 1),
                        )
                u_cols = ucolsp.tile([P, 2 * GSEG], BF16, tag="ucols")
                nc.vector.tensor_copy(u_cols[:], pu[:])
                u_cols_l[g] = u_cols

                # c row: b_i . h + b_k (bias again via ones-row matmul)
                puc = chainp.tile([1, GSEG], F32, tag="chainc")
                for j in range(2):
                    nc.tensor.matmul(
                        out=puc[:],
                        lhsT=w_ext_sb[:, j * W1 + H: j * W1 + H + 1],
                        rhs=hT[:, j * GSEG:(j + 1) * GSEG],
                        start=(j == 0), stop=False,
                    )
                nc.tensor.matmul(
                    out=puc[:], lhsT=b_k_sb[:1, :1], rhs=ones1[:1, :GSEG],
                    start=False, stop=True,
                )
                uc_sb = grp.tile([1, GSEG], F32, tag="ucsb")
                nc.vector.tensor_copy(uc_sb[:], puc[:])
                uc_l[g] = uc_sb

            def emit_cseed(g):
                """Allocate group g's PSUM logits tile.  Column layout:
                [0:16] pos (by segment), [16:96] neg as 16*r + s_local
                (host unpermutes)."""
                pd = dotp.tile([P, GRP_W], F32, tag="dot")
                pd_l[g] = pd

            def emit_dots(g, tile, coff, blocks):
                """blocks: list of (pd_col, block_in_call, sloc).  Each
                column is a 3-matmul accumulation group: ones x c seed,
                then the two feature-chunk dot matmuls."""
                pd = pd_l[g]
                u_cols = u_cols_l[g]
                uc_sb = uc_l[g]
                for pcol, b, sloc in blocks:
                    nc.tensor.matmul(
                        out=pd[:, pcol:pcol + 1],
                        lhsT=ones1[:1, :P],
                        rhs=uc_sb[:1, sloc:sloc + 1],
                        start=True, stop=False,
                    )
                    for c in range(2):
                        nc.tensor.matmul(
                            out=pd[:, pcol:pcol + 1],
                            lhsT=tile[:, coff + c * CALL_IDX + b * P:
                                      coff + c * CALL_IDX + (b + 1) * P],
                            rhs=u_cols[:, c * GSEG + sloc:
                                       c * GSEG + sloc + 1],
                            start=False, stop=(c == 1),
                        )

            def emit_pd_copy(g):
                nc.vector.tensor_copy(
                    logits_sb[:, g * GRP_W:(g + 1) * GRP_W], pd_l[g][:])

            # ---- prologue ----
            emit_pos_group(0)
            emit_pos_group(1)
            # consts ride on the ACT lane ahead of its neg loads
            nc.scalar.dma_start(w_iT_sb[:], w_iT[:, :, :])
            nc.scalar.dma_start(wbT_sb[:], wbT[:, :, :])
            nc.scalar.dma_start(w_ext_sb[:], w_ext[:, :, :])
            nc.scalar.dma_start(b_iR_sb[:], b_iR[:, :])
            nc.scalar.dma_start(b_k_sb[:], b_k[:, :])
            emit_prep(0)

            # ---- main loop, prep pipelined one group ahead ----
            for g in range(NGRP):
                emit_cseed(g)
                # pos dots of group g
                for cal in range(2):
                    emit_dots(g, pos_gtiles[g], cal * CALL_W,
                              [(cal * GB + b, b, cal * GB + b)
                               for b in range(GB)])
                for i in range(NEG_PER_GRP):
                    gi = g * NEG_PER_GRP + i
                    emit_neg(gi)
                    blocks = []
                    for b in range(GB):
                        ql = i * GB + b             # 0..79 within group
                        sl, r = ql // NEG_RATIO, ql % NEG_RATIO
                        blocks.append((GSEG + 16 * r + sl, b, sl))
                    emit_dots(g, neg_tiles[gi], 0, blocks)
                    if i == 2 and g + 1 < NGRP:
                        emit_prep(g + 1)
                    if i == 5 and g + 2 < NGRP:
                        emit_pos_group(g + 2)
                    if i == 7 and g > 0:
                        emit_pd_copy(g - 1)

            emit_pd_copy(NGRP - 1)
            nc.sync.dma_start(logits_d[:, :], logits_sb[:])

    nc.compile()
    return nc


def get_module() -> bass.Bass:
    global _CACHED
    if _CACHED is None:
        _CACHED = _build_module()
    return _CACHED


def make_in_maps(inputs: dict) -> list[dict]:
    emb = np.ascontiguousarray(np.asarray(inputs["embedding"], dtype=np.float32))
    gs = np.asarray(inputs["grid_sizes"]).astype(np.int64)
    pos_s = np.asarray(inputs["pos_samples"]).astype(np.int64)
    neg_s = np.asarray(inputs["neg_samples"]).astype(np.int64)
    W_i = np.asarray(inputs["W_i"], dtype=np.float32)
    b_i = np.asarray(inputs["b_i"], dtype=np.float32)
    Wb = np.asarray(inputs["W_k"], dtype=np.float32)[0]
    b_kv = np.asarray(inputs["b_k"], dtype=np.float32)

    if not (gs.shape == (N_SEG,) and np.all(gs == SEG_SZ)):
        raise RuntimeError("kernel assumes grid_sizes == 128 everywhere")
    assert pos_s.shape == (N_POS,) and neg_s.shape == (N_NEG,)

    emb_bf = emb.astype(ml_dtypes.bfloat16)

    # mean = sum/seg_sz folded into the first chain matmul's weights
    w_iT_np = np.ascontiguousarray(
        (W_i / float(SEG_SZ)).T.reshape(2, P, H).transpose(1, 0, 2))
    wbT_np = np.ascontiguousarray(
        Wb.T.reshape(2, P, H).transpose(1, 0, 2))
    W_ext = np.concatenate([W_i, b_i[:, None]], axis=1)        # [256, 257]
    w_ext_np = np.ascontiguousarray(
        W_ext.reshape(2, P, H + 1).transpose(1, 0, 2))
    b_iR_np = b_i.reshape(1, H)
    b_k_np = b_kv.reshape(1, 1)

    in_maps = []
    for k in range(N_CORES):
        # processing order: pos rows then neg rows of this core, staged in
        # the device's transposed block layout:
        # rows[p, ci*2048 + c*1024 + n] = emb[full[ci*1024+n], c*128+p]
        full = np.concatenate([
            pos_s[k * POS_PC:(k + 1) * POS_PC],
            neg_s[k * NEG_PC:(k + 1) * NEG_PC],
        ])
        g = emb_bf[full]                       # [98304, 256]
        rows_np = np.ascontiguousarray(
            g.reshape(N_CALLS, CALL_IDX, 2, P).transpose(3, 0, 2, 1)
            .reshape(P, N_CALLS * CALL_W))
        in_maps.append({
            "rows": rows_np,
            "w_iT": w_iT_np,
            "wbT": wbT_np,
            "w_ext": w_ext_np,
            "b_iR": b_iR_np,
            "b_k": b_k_np,
        })
    return in_maps


def assemble_output(core_outs: list[np.ndarray]) -> np.ndarray:
    pos_parts, neg_parts = [], []
    for k in range(N_CORES):
        o = np.asarray(core_outs[k])
        assert o.shape == (P, TOT_BLK)
        o3 = o.reshape(P, NGRP, GRP_W)
        # pos: col 16*g + sl -> block 16*g + sl -> rows [block, p]
        pos_parts.append(np.ascontiguousarray(
            o3[:, :, :GSEG].transpose(1, 2, 0)).ravel())
        # neg: col 16 + 16*r + sl of group g -> q = 80*g + 5*sl + r
        neg_parts.append(np.ascontiguousarray(
            o3[:, :, GSEG:].reshape(P, NGRP, NEG_RATIO, GSEG)
            .transpose(1, 3, 2, 0)).ravel())
    return np.concatenate(pos_parts + neg_parts).astype(np.float32)


def kernel(**inputs) -> np.ndarray:
    nc = get_module()
    in_maps = make_in_maps(inputs)
    res = bass_utils.run_bass_kernel_spmd(
        nc, in_maps, core_ids=list(range(N_CORES)))
    return assemble_output([r["logits"] for r in res.results])


# revision 19
# speedup vs baseline: 2.7285x; 1.1074x over previous
"""Trainium2 Bass kernel for nn_Discriminator (segment_reduce, 8 cores).

Math (collapsed form of the reference):
  The reference projects the full embedding table (emb = E @ W_i.T + b_i),
  gathers pos/neg rows, does a segment-mean over pos rows, and scores each
  row with a bilinear form against its segment embedding.  Everything is
  linear, so it collapses to operations on RAW embedding rows:

    m[s]     = mean of raw E rows of segment s's pos samples        [256]
    grid[s]  = W_i m[s] + b_i
    h[s]     = Wb grid[s]                  (Wb = W_k[0])
    u[s]     = W_i^T h[s];   c[s] = b_i . h[s] + b_k
    logit[n] = E[idx[n]] . u[seg(n)] + c[seg(n)]

  The 1/seg_size mean scaling is folded into a host-prescaled W_i; the
  b_i / b_k / c biases are folded into PE matmuls (ones-row outer
  products), so the chain needs no per-element bias ops at all.

Sharding: data-parallel over samples, segments kept whole per core
(core k owns segments [k*128, (k+1)*128)).  Fully local, no collectives.

Device pipeline per core:
  - The host stages each core's sampled rows (pos then neg, in processing
    order) as bf16 in a feature-transposed block layout; the device
    streams them with large sequential DMAs split across THREE issuing
    engines (SP / Activation HWDGE + gpsimd SWDGE) so the transfers
    pipeline three-wide (~50 MB/core of traffic).
    Layout: rows[p, ci*2048 + c*1024 + n] = feature c*128+p of row n.
  - Segment sums: one 3-D DVE tensor_reduce per (pos call, chunk)
    reduces the innermost 128 rows for 8 segments at once.
  - The tiny u-chain runs per group of 16 segments, software-pipelined
    one group ahead; u stays as COLUMNS so it feeds the dots directly.
  - Per 128-row block: 2 accumulating PE matmuls (lhsT = transposed rows
    chunk [128x128], rhs = u column [128x1]) produce the block's logits
    in a per-group [128,96] PSUM tile whose columns were pre-seeded with
    the c bias by 6 ones-row matmuls; one DVE copy per group moves the
    finished logits to SBUF.  Neg columns are (r,s)-reordered inside the
    group so every c seed is a contiguous 16-column matmul; the host
    unpermutes when assembling the output.
"""

import numpy as np
import ml_dtypes

import concourse.bass as bass
import concourse.bacc as bacc
import concourse.mybir as mybir
from concourse import bass_utils
from concourse.tile import TileContext

F32 = mybir.dt.float32
BF16 = mybir.dt.bfloat16

N_NODES = 200000
H = 256
N_SEG = 1024
SEG_SZ = 128          # rows per segment (asserted at runtime)
N_POS = N_SEG * SEG_SZ          # 131072
NEG_RATIO = 5
N_NEG = N_POS * NEG_RATIO       # 655360
N_CORES = 8

SEG_PC = N_SEG // N_CORES       # 128 segments per core
POS_PC = N_POS // N_CORES       # 16384
NEG_PC = N_NEG // N_CORES       # 81920
P = 128
POS_BLK = POS_PC // P           # 128 blocks (block == segment for pos)
NEG_BLK = NEG_PC // P           # 640 blocks (5 consecutive per segment)
TOT_BLK = POS_BLK + NEG_BLK     # 768 logit columns

GB = 8                          # blocks per "call" (1024 rows)
CALL_IDX = GB * P               # 1024 rows per call
CALL_W = 2 * CALL_IDX           # 2048 bf16 columns per call tile
NEG_BUFS = 12                   # in-flight neg tiles
GSEG = 16                       # segments per u-chain group
NGRP = SEG_PC // GSEG           # 8 groups
POS_CALLS = POS_BLK // GB       # 16 (2 per group)
NEG_CALLS = NEG_BLK // GB       # 80 (10 per group)
N_CALLS = POS_CALLS + NEG_CALLS # 96
NEG_PER_GRP = NEG_CALLS // NGRP # 10
GRP_W = GSEG + NEG_PER_GRP * GB # 96 logit columns per group

# neg-load lane assignment: ACT (scalar) carries the consts and no pos
# loads, so it takes ~4 of each 10; one swap at the end evens the totals
# to A31 / S24 / P25.
NEG_LANES = (["scalar", "sync", "gpsimd", "scalar", "sync",
              "gpsimd", "scalar", "sync", "gpsimd", "scalar"] * NGRP)
NEG_LANES[-1] = "gpsimd"

_CACHED = None


def _build_module() -> bass.Bass:
    # Bacc (not raw Bass): its compile() pass splits multi-sem waits into
    # event semaphores — walrus rejects >1 sync wait per instruction.
    nc = bacc.Bacc("TRN2", target_bir_lowering=False, debug=False)

    rows = nc.dram_tensor("rows", [P, N_CALLS * CALL_W], BF16,
                          kind="ExternalInput")
    # w_iT[p, j, f'] = (W_i/seg_sz).T[j*128+p, f']  (lhsT tiles, G = W_i @ M)
    w_iT = nc.dram_tensor("w_iT", [P, 2, H], F32, kind="ExternalInput")
    # wbT[p, j, d]  = Wb.T[j*128+p, d]        (lhsT tiles for H = Wb @ G)
    wbT = nc.dram_tensor("wbT", [P, 2, H], F32, kind="ExternalInput")
    # w_ext[p, j, m] = [W_i | b_i][j*128+p, m]  (lhsT tiles for U~ = W_ext^T H)
    w_ext = nc.dram_tensor("w_ext", [P, 2, H + 1], F32, kind="ExternalInput")
    b_iR = nc.dram_tensor("b_iR", [1, H], F32, kind="ExternalInput")
    b_k = nc.dram_tensor("b_k", [1, 1], F32, kind="ExternalInput")
    logits_d = nc.dram_tensor("logits", [P, TOT_BLK], F32, kind="ExternalOutput")

    W1 = H + 1

    with TileContext(nc) as tc:
        with (
            tc.tile_pool(name="const", bufs=1) as const,
            tc.tile_pool(name="grp", bufs=2) as grp,
            tc.tile_pool(name="ucols", bufs=3) as ucolsp,
            tc.tile_pool(name="pospool", bufs=POS_CALLS) as pospool,
            tc.tile_pool(name="negpool", bufs=NEG_BUFS) as negpool,
            tc.tile_pool(name="chain", bufs=2, space="PSUM") as chainp,
            tc.tile_pool(name="dot", bufs=3, space="PSUM") as dotp,
        ):
            # ---- tiles ----
            ones1 = const.tile([1, P], F32, tag="ones1")
            nc.gpsimd.memset(ones1[:], 1.0)

            w_iT_sb = const.tile([P, 2 * H], F32, tag="wiT")
            wbT_sb = const.tile([P, 2 * H], F32, tag="wbT")
            w_ext_sb = const.tile([P, 2 * W1], F32, tag="wext")
            b_iR_sb = const.tile([1, H], F32, tag="biR")
            b_k_sb = const.tile([1, 1], F32, tag="bk")
            logits_sb = const.tile([P, TOT_BLK], F32, tag="logits")

            pos_tiles = [None] * POS_CALLS
            neg_tiles = [None] * NEG_CALLS
            u_cols_l = [None] * NGRP
            uc_l = [None] * NGRP
            pd_l = [None] * NGRP

            pos_lane = ["sync", "gpsimd"]

            def emit_pos_group(g):
                for cal in range(2):
                    pt = pospool.tile([P, CALL_W], BF16, tag="pos")
                    pos_tiles[2 * g + cal] = pt
                    getattr(nc, pos_lane[cal]).dma_start(
                        pt[:], rows[:, (2 * g + cal) * CALL_W:
                                    (2 * g + cal + 1) * CALL_W])

            def emit_neg(gi):
                t = negpool.tile([P, CALL_W], BF16, tag="neg")
                neg_tiles[gi] = t
                getattr(nc, NEG_LANES[gi]).dma_start(
                    t[:], rows[:, (POS_CALLS + gi) * CALL_W:
                               (POS_CALLS + gi + 1) * CALL_W])

            def emit_prep(g):
                """Segment sums + u-chain for group g (pos tile must be
                in flight); produces u_cols_l[g] (bf16) and uc_l[g]."""
                mT = grp.tile([P, 2 * GSEG], F32, tag="mT")
                for cal in range(2):
                    for c in range(2):
                        nc.vector.tensor_reduce(
                            out=mT[:, c * GSEG + cal * GB:
                                   c * GSEG + cal * GB + GB],
                            in_=pos_tiles[2 * g + cal][
                                :, c * CALL_IDX:(c + 1) * CALL_IDX]
                                .rearrange("p (s n) -> p s n", s=GB),
                            op=mybir.AluOpType.add,
                            axis=mybir.AxisListType.X,
                        )

                # G_T = (W_i/seg_sz) @ M_T + b_i (bias via ones-row matmul)
                pg = chainp.tile([P, 2 * GSEG], F32, tag="chain")
                for t in range(2):
                    for j in range(2):
                        nc.tensor.matmul(
                            out=pg[:, t * GSEG:(t + 1) * GSEG],
                            lhsT=w_iT_sb[:, j * H + t * P: j * H + t * P + P],
                            rhs=mT[:, j * GSEG:(j + 1) * GSEG],
                            start=(j == 0), stop=False,
                        )
                    nc.tensor.matmul(
                        out=pg[:, t * GSEG:(t + 1) * GSEG],
                        lhsT=b_iR_sb[:1, t * P:(t + 1) * P],
                        rhs=ones1[:1, :GSEG],
                        start=False, stop=True,
                    )
                gT = grp.tile([P, 2 * GSEG], F32, tag="gT")
                nc.vector.tensor_copy(gT[:], pg[:])

                # H_T = Wb @ G_T
                ph = chainp.tile([P, 2 * GSEG], F32, tag="chain")
                for t in range(2):
                    for j in range(2):
                        nc.tensor.matmul(
                            out=ph[:, t * GSEG:(t + 1) * GSEG],
                            lhsT=wbT_sb[:, j * H + t * P: j * H + t * P + P],
                            rhs=gT[:, j * GSEG:(j + 1) * GSEG],
                            start=(j == 0), stop=(j == 1),
                        )
                hT = grp.tile([P, 2 * GSEG], F32, tag="hT")
                nc.vector.tensor_copy(hT[:], ph[:])

                # U~_T = [W_i | b_i]^T @ H_T, kept as bf16 COLUMNS
                pu = chainp.tile([P, 2 * GSEG], F32, tag="chain")
                for t in range(2):
                    for j in range(2):
                        nc.tensor.matmul(
                            out=pu[:, t * GSEG:(t + 1) * GSEG],
                            lhsT=w_ext_sb[:, j * W1 + t * P: j * W1 + t * P + P],
                            rhs=hT[:, j * GSEG:(j + 1) * GSEG],
                            start=(j == 0), stop=(j == 1),
                        )
                u_cols = ucolsp.tile([P, 2 * GSEG], BF16, tag="ucols")
                nc.vector.tensor_copy(u_cols[:], pu[:])
                u_cols_l[g] = u_cols

                # c row: b_i . h + b_k (bias again via ones-row matmul)
                puc = chainp.tile([1, GSEG], F32, tag="chainc")
                for j in range(2):
                    nc.tensor.matmul(
                        out=puc[:],
                        lhsT=w_ext_sb[:, j * W1 + H: j * W1 + H + 1],
                        rhs=hT[:, j * GSEG:(j + 1) * GSEG],
                        start=(j == 0), stop=False,
                    )
                nc.tensor.matmul(
                    out=puc[:], lhsT=b_k_sb[:1, :1], rhs=ones1[:1, :GSEG],
                    start=False, stop=True,
                )
                uc_sb = grp.tile([1, GSEG], F32, tag="ucsb")
                nc.vector.tensor_copy(uc_sb[:], puc[:])
                uc_l[g] = uc_sb

            def emit_cseed(g):
                """Allocate group g's PSUM logits tile.  Column layout:
                [0:16] pos (by segment), [16:96] neg as 16*r + s_local
                (host unpermutes)."""
                pd = dotp.tile([P, GRP_W], F32, tag="dot")
                pd_l[g] = pd

            def emit_dots(g, tile, coff, blocks):
                """blocks: list of (pd_col, block_in_call, sloc).  Each
                column is a 3-matmul accumulation group: ones x c seed,
                then the two feature-chunk dot matmuls."""
                pd = pd_l[g]
                u_cols = u_cols_l[g]
                uc_sb = uc_l[g]
                for pcol, b, sloc in blocks:
                    nc.tensor.matmul(
                        out=pd[:, pcol:pcol + 1],
                        lhsT=ones1[:1, :P],
                        rhs=uc_sb[:1, sloc:sloc + 1],
                        start=True, stop=False,
                    )
                    for c in range(2):
                        nc.tensor.matmul(
                            out=pd[:, pcol:pcol + 1],
                            lhsT=tile[:, coff + c * CALL_IDX + b * P:
                                      coff + c * CALL_IDX + (b + 1) * P],
                            rhs=u_cols[:, c * GSEG + sloc:
                                       c * GSEG + sloc + 1],
                            start=False, stop=(c == 1),
                        )

            def emit_pd_copy(g):
                nc.vector.tensor_copy(
                    logits_sb[:, g * GRP_W:(g + 1) * GRP_W], pd_l[g][:])

            # ---- prologue ----
            emit_pos_group(0)
            emit_pos_group(1)
            # consts ride on the ACT lane ahead of its neg loads
            nc.scalar.dma_start(w_iT_sb[:], w_iT[:, :, :])
            nc.scalar.dma_start(wbT_sb[:], wbT[:, :, :])
            nc.scalar.dma_start(w_ext_sb[:], w_ext[:, :, :])
            nc.scalar.dma_start(b_iR_sb[:], b_iR[:, :])
            nc.scalar.dma_start(b_k_sb[:], b_k[:, :])
            emit_prep(0)

            # ---- main loop, prep pipelined one group ahead ----
            for g in range(NGRP):
                emit_cseed(g)
                # pos dots of group g
                for cal in range(2):
                    emit_dots(g, pos_tiles[2 * g + cal], 0,
                              [(cal * GB + b, b, cal * GB + b)
                               for b in range(GB)])
                for i in range(NEG_PER_GRP):
                    gi = g * NEG_PER_GRP + i
                    emit_neg(gi)
                    blocks = []
                    for b in range(GB):
                        ql = i * GB + b             # 0..79 within group
                        sl, r = ql // NEG_RATIO, ql % NEG_RATIO
                        blocks.append((GSEG + 16 * r + sl, b, sl))
                    emit_dots(g, neg_tiles[gi], 0, blocks)
                    if i == 2 and g + 1 < NGRP:
                        emit_prep(g + 1)
                    if i == 5 and g + 2 < NGRP:
                        emit_pos_group(g + 2)
                    if i == 7 and g > 0:
                        emit_pd_copy(g - 1)

            emit_pd_copy(NGRP - 1)
            nc.sync.dma_start(logits_d[:, :], logits_sb[:])

    nc.compile()
    return nc


def get_module() -> bass.Bass:
    global _CACHED
    if _CACHED is None:
        _CACHED = _build_module()
    return _CACHED


def make_in_maps(inputs: dict) -> list[dict]:
    emb = np.ascontiguousarray(np.asarray(inputs["embedding"], dtype=np.float32))
    gs = np.asarray(inputs["grid_sizes"]).astype(np.int64)
    pos_s = np.asarray(inputs["pos_samples"]).astype(np.int64)
    neg_s = np.asarray(inputs["neg_samples"]).astype(np.int64)
    W_i = np.asarray(inputs["W_i"], dtype=np.float32)
    b_i = np.asarray(inputs["b_i"], dtype=np.float32)
    Wb = np.asarray(inputs["W_k"], dtype=np.float32)[0]
    b_kv = np.asarray(inputs["b_k"], dtype=np.float32)

    if not (gs.shape == (N_SEG,) and np.all(gs == SEG_SZ)):
        raise RuntimeError("kernel assumes grid_sizes == 128 everywhere")
    assert pos_s.shape == (N_POS,) and neg_s.shape == (N_NEG,)

    emb_bf = emb.astype(ml_dtypes.bfloat16)

    # mean = sum/seg_sz folded into the first chain matmul's weights
    w_iT_np = np.ascontiguousarray(
        (W_i / float(SEG_SZ)).T.reshape(2, P, H).transpose(1, 0, 2))
    wbT_np = np.ascontiguousarray(
        Wb.T.reshape(2, P, H).transpose(1, 0, 2))
    W_ext = np.concatenate([W_i, b_i[:, None]], axis=1)        # [256, 257]
    w_ext_np = np.ascontiguousarray(
        W_ext.reshape(2, P, H + 1).transpose(1, 0, 2))
    b_iR_np = b_i.reshape(1, H)
    b_k_np = b_kv.reshape(1, 1)

    in_maps = []
    for k in range(N_CORES):
        # processing order: pos rows then neg rows of this core, staged in
        # the device's transposed block layout:
        # rows[p, ci*2048 + c*1024 + n] = emb[full[ci*1024+n], c*128+p]
        full = np.concatenate([
            pos_s[k * POS_PC:(k + 1) * POS_PC],
            neg_s[k * NEG_PC:(k + 1) * NEG_PC],
        ])
        g = emb_bf[full]                       # [98304, 256]
        rows_np = np.ascontiguousarray(
            g.reshape(N_CALLS, CALL_IDX, 2, P).transpose(3, 0, 2, 1)
            .reshape(P, N_CALLS * CALL_W))
        in_maps.append({
            "rows": rows_np,
            "w_iT": w_iT_np,
            "wbT": wbT_np,
            "w_ext": w_ext_np,
            "b_iR": b_iR_np,
            "b_k": b_k_np,
        })
    return in_maps


def assemble_output(core_outs: list[np.ndarray]) -> np.ndarray:
    pos_parts, neg_parts = [], []
    for k in range(N_CORES):
        o = np.asarray(core_outs[k])
        assert o.shape == (P, TOT_BLK)
        o3 = o.reshape(P, NGRP, GRP_W)
        # pos: col 16*g + sl -> block 16*g + sl -> rows [block, p]
        pos_parts.append(np.ascontiguousarray(
            o3[:, :, :GSEG].transpose(1, 2, 0)).ravel())
        # neg: col 16 + 16*r + sl of group g -> q = 80*g + 5*sl + r
        neg_parts.append(np.ascontiguousarray(
            o3[:, :, GSEG:].reshape(P, NGRP, NEG_RATIO, GSEG)
            .transpose(1, 3, 2, 0)).ravel())
    return np.concatenate(pos_parts + neg_parts).astype(np.float32)


def kernel(**inputs) -> np.ndarray:
    nc = get_module()
    in_maps = make_in_maps(inputs)
    res = bass_utils.run_bass_kernel_spmd(
        nc, in_maps, core_ids=list(range(N_CORES)))
    return assemble_output([r["logits"] for r in res.results])


# revision 21
# speedup vs baseline: 2.7818x; 1.0195x over previous
"""Trainium2 Bass kernel for nn_Discriminator (segment_reduce, 8 cores).

Math (collapsed form of the reference):
  The reference projects the full embedding table (emb = E @ W_i.T + b_i),
  gathers pos/neg rows, does a segment-mean over pos rows, and scores each
  row with a bilinear form against its segment embedding.  Everything is
  linear, so it collapses to operations on RAW embedding rows:

    m[s]     = mean of raw E rows of segment s's pos samples        [256]
    grid[s]  = W_i m[s] + b_i
    h[s]     = Wb grid[s]                  (Wb = W_k[0])
    u[s]     = W_i^T h[s];   c[s] = b_i . h[s] + b_k
    logit[n] = E[idx[n]] . u[seg(n)] + c[seg(n)]

  The 1/seg_size mean scaling is folded into a host-prescaled W_i; the
  b_i / b_k / c biases are folded into PE matmuls (ones-row outer
  products), so the chain needs no per-element bias ops at all.

Sharding: data-parallel over samples, segments kept whole per core
(core k owns segments [k*128, (k+1)*128)).  Fully local, no collectives.

Device pipeline per core:
  - The host stages each core's sampled rows (pos then neg, in processing
    order) as bf16 in a feature-transposed block layout; the device
    streams them with large sequential DMAs split across THREE issuing
    engines (SP / Activation HWDGE + gpsimd SWDGE) so the transfers
    pipeline three-wide (~50 MB/core of traffic).
    Layout: rows[p, ci*2048 + c*1024 + n] = feature c*128+p of row n.
  - Segment sums: one 3-D DVE tensor_reduce per (pos call, chunk)
    reduces the innermost 128 rows for 8 segments at once.
  - The tiny u-chain runs per group of 16 segments, software-pipelined
    one group ahead; u stays as COLUMNS so it feeds the dots directly.
  - Per 128-row block: 2 accumulating PE matmuls (lhsT = transposed rows
    chunk [128x128], rhs = u column [128x1]) produce the block's logits
    in a per-group [128,96] PSUM tile whose columns were pre-seeded with
    the c bias by 6 ones-row matmuls; one DVE copy per group moves the
    finished logits to SBUF.  Neg columns are (r,s)-reordered inside the
    group so every c seed is a contiguous 16-column matmul; the host
    unpermutes when assembling the output.
"""

import numpy as np
import ml_dtypes

import concourse.bass as bass
import concourse.bacc as bacc
import concourse.mybir as mybir
from concourse import bass_utils
from concourse.tile import TileContext

F32 = mybir.dt.float32
BF16 = mybir.dt.bfloat16

N_NODES = 200000
H = 256
N_SEG = 1024
SEG_SZ = 128          # rows per segment (asserted at runtime)
N_POS = N_SEG * SEG_SZ          # 131072
NEG_RATIO = 5
N_NEG = N_POS * NEG_RATIO       # 655360
N_CORES = 8

SEG_PC = N_SEG // N_CORES       # 128 segments per core
POS_PC = N_POS // N_CORES       # 16384
NEG_PC = N_NEG // N_CORES       # 81920
P = 128
POS_BLK = POS_PC // P           # 128 blocks (block == segment for pos)
NEG_BLK = NEG_PC // P           # 640 blocks (5 consecutive per segment)
TOT_BLK = POS_BLK + NEG_BLK     # 768 logit columns

GB = 8                          # blocks per "call" (1024 rows)
CALL_IDX = GB * P               # 1024 rows per call
CALL_W = 2 * CALL_IDX           # 2048 bf16 columns per call tile
NEG_BUFS = 14                   # in-flight neg tiles
GSEG = 16                       # segments per u-chain group
NGRP = SEG_PC // GSEG           # 8 groups
POS_CALLS = POS_BLK // GB       # 16 (2 per group)
NEG_CALLS = NEG_BLK // GB       # 80 (10 per group)
N_CALLS = POS_CALLS + NEG_CALLS # 96
NEG_PER_GRP = NEG_CALLS // NGRP # 10
GRP_W = GSEG + NEG_PER_GRP * GB # 96 logit columns per group

# neg-load lane assignment: ACT (scalar) carries the consts and no pos
# loads, so it takes ~4 of each 10; one swap at the end evens the totals
# to A31 / S24 / P25.
NEG_LANES = (["scalar", "sync", "gpsimd", "scalar", "sync",
              "gpsimd", "scalar", "sync", "gpsimd", "scalar"] * NGRP)
NEG_LANES[-1] = "gpsimd"

_CACHED = None


def _build_module() -> bass.Bass:
    # Bacc (not raw Bass): its compile() pass splits multi-sem waits into
    # event semaphores — walrus rejects >1 sync wait per instruction.
    nc = bacc.Bacc("TRN2", target_bir_lowering=False, debug=False)

    rows = nc.dram_tensor("rows", [P, N_CALLS * CALL_W], BF16,
                          kind="ExternalInput")
    # w_iT[p, j, f'] = (W_i/seg_sz).T[j*128+p, f']  (lhsT tiles, G = W_i @ M)
    w_iT = nc.dram_tensor("w_iT", [P, 2, H], F32, kind="ExternalInput")
    # wbT[p, j, d]  = Wb.T[j*128+p, d]        (lhsT tiles for H = Wb @ G)
    wbT = nc.dram_tensor("wbT", [P, 2, H], F32, kind="ExternalInput")
    # w_ext[p, j, m] = [W_i | b_i][j*128+p, m]  (lhsT tiles for U~ = W_ext^T H)
    w_ext = nc.dram_tensor("w_ext", [P, 2, H + 1], F32, kind="ExternalInput")
    b_iR = nc.dram_tensor("b_iR", [1, H], F32, kind="ExternalInput")
    b_k = nc.dram_tensor("b_k", [1, 1], F32, kind="ExternalInput")
    logits_d = nc.dram_tensor("logits", [P, TOT_BLK], F32, kind="ExternalOutput")

    W1 = H + 1

    with TileContext(nc) as tc:
        with (
            tc.tile_pool(name="const", bufs=1) as const,
            tc.tile_pool(name="grp", bufs=2) as grp,
            tc.tile_pool(name="ucols", bufs=3) as ucolsp,
            tc.tile_pool(name="pospool", bufs=POS_CALLS) as pospool,
            tc.tile_pool(name="negpool", bufs=NEG_BUFS) as negpool,
            tc.tile_pool(name="chain", bufs=2, space="PSUM") as chainp,
            tc.tile_pool(name="dot", bufs=3, space="PSUM") as dotp,
        ):
            # ---- tiles ----
            ones1 = const.tile([1, P], F32, tag="ones1")
            nc.gpsimd.memset(ones1[:], 1.0)

            w_iT_sb = const.tile([P, 2 * H], F32, tag="wiT")
            wbT_sb = const.tile([P, 2 * H], F32, tag="wbT")
            w_ext_sb = const.tile([P, 2 * W1], F32, tag="wext")
            b_iR_sb = const.tile([1, H], F32, tag="biR")
            b_k_sb = const.tile([1, 1], F32, tag="bk")
            logits_sb = const.tile([P, TOT_BLK], F32, tag="logits")

            pos_tiles = [None] * POS_CALLS
            neg_tiles = [None] * NEG_CALLS
            u_cols_l = [None] * NGRP
            uc_l = [None] * NGRP
            pd_l = [None] * NGRP

            pos_lane = ["sync", "gpsimd"]

            def emit_pos_group(g):
                for cal in range(2):
                    pt = pospool.tile([P, CALL_W], BF16, tag="pos")
                    pos_tiles[2 * g + cal] = pt
                    getattr(nc, pos_lane[cal]).dma_start(
                        pt[:], rows[:, (2 * g + cal) * CALL_W:
                                    (2 * g + cal + 1) * CALL_W])

            def emit_neg(gi):
                t = negpool.tile([P, CALL_W], BF16, tag="neg")
                neg_tiles[gi] = t
                getattr(nc, NEG_LANES[gi]).dma_start(
                    t[:], rows[:, (POS_CALLS + gi) * CALL_W:
                               (POS_CALLS + gi + 1) * CALL_W])

            def emit_prep(g):
                """Segment sums + u-chain for group g (pos tile must be
                in flight); produces u_cols_l[g] (bf16) and uc_l[g]."""
                mT = grp.tile([P, 2 * GSEG], F32, tag="mT")
                for cal in range(2):
                    for c in range(2):
                        nc.vector.tensor_reduce(
                            out=mT[:, c * GSEG + cal * GB:
                                   c * GSEG + cal * GB + GB],
                            in_=pos_tiles[2 * g + cal][
                                :, c * CALL_IDX:(c + 1) * CALL_IDX]
                                .rearrange("p (s n) -> p s n", s=GB),
                            op=mybir.AluOpType.add,
                            axis=mybir.AxisListType.X,
                        )

                # G_T = (W_i/seg_sz) @ M_T + b_i (bias via ones-row matmul)
                pg = chainp.tile([P, 2 * GSEG], F32, tag="chain")
                for t in range(2):
                    for j in range(2):
                        nc.tensor.matmul(
                            out=pg[:, t * GSEG:(t + 1) * GSEG],
                            lhsT=w_iT_sb[:, j * H + t * P: j * H + t * P + P],
                            rhs=mT[:, j * GSEG:(j + 1) * GSEG],
                            start=(j == 0), stop=False,
                        )
                    nc.tensor.matmul(
                        out=pg[:, t * GSEG:(t + 1) * GSEG],
                        lhsT=b_iR_sb[:1, t * P:(t + 1) * P],
                        rhs=ones1[:1, :GSEG],
                        start=False, stop=True,
                    )
                gT = grp.tile([P, 2 * GSEG], F32, tag="gT")
                nc.vector.tensor_copy(gT[:], pg[:])

                # H_T = Wb @ G_T
                ph = chainp.tile([P, 2 * GSEG], F32, tag="chain")
                for t in range(2):
                    for j in range(2):
                        nc.tensor.matmul(
                            out=ph[:, t * GSEG:(t + 1) * GSEG],
                            lhsT=wbT_sb[:, j * H + t * P: j * H + t * P + P],
                            rhs=gT[:, j * GSEG:(j + 1) * GSEG],
                            start=(j == 0), stop=(j == 1),
                        )
                hT = grp.tile([P, 2 * GSEG], F32, tag="hT")
                nc.vector.tensor_copy(hT[:], ph[:])

                # U~_T = [W_i | b_i]^T @ H_T, kept as bf16 COLUMNS
                pu = chainp.tile([P, 2 * GSEG], F32, tag="chain")
                for t in range(2):
                    for j in range(2):
                        nc.tensor.matmul(
                            out=pu[:, t * GSEG:(t + 1) * GSEG],
                            lhsT=w_ext_sb[:, j * W1 + t * P: j * W1 + t * P + P],
                            rhs=hT[:, j * GSEG:(j + 1) * GSEG],
                            start=(j == 0), stop=(j == 1),
                        )
                u_cols = ucolsp.tile([P, 2 * GSEG], BF16, tag="ucols")
                nc.vector.tensor_copy(u_cols[:], pu[:])
                u_cols_l[g] = u_cols

                # c row: b_i . h + b_k (bias again via ones-row matmul)
                puc = chainp.tile([1, GSEG], F32, tag="chainc")
                for j in range(2):
                    nc.tensor.matmul(
                        out=puc[:],
                        lhsT=w_ext_sb[:, j * W1 + H: j * W1 + H + 1],
                        rhs=hT[:, j * GSEG:(j + 1) * GSEG],
                        start=(j == 0), stop=False,
                    )
                nc.tensor.matmul(
                    out=puc[:], lhsT=b_k_sb[:1, :1], rhs=ones1[:1, :GSEG],
                    start=False, stop=True,
                )
                uc_sb = grp.tile([1, GSEG], F32, tag="ucsb")
                nc.vector.tensor_copy(uc_sb[:], puc[:])
                uc_l[g] = uc_sb

            def emit_cseed(g):
                """Allocate group g's PSUM logits tile.  Column layout:
                [0:16] pos (by segment), [16:96] neg as 16*r + s_local
                (host unpermutes)."""
                pd = dotp.tile([P, GRP_W], F32, tag="dot")
                pd_l[g] = pd

            def emit_dots(g, tile, coff, blocks):
                """blocks: list of (pd_col, block_in_call, sloc).  Each
                column is a 3-matmul accumulation group: ones x c seed,
                then the two feature-chunk dot matmuls."""
                pd = pd_l[g]
                u_cols = u_cols_l[g]
                uc_sb = uc_l[g]
                for pcol, b, sloc in blocks:
                    nc.tensor.matmul(
                        out=pd[:, pcol:pcol + 1],
                        lhsT=ones1[:1, :P],
                        rhs=uc_sb[:1, sloc:sloc + 1],
                        start=True, stop=False,
                    )
                    for c in range(2):
                        nc.tensor.matmul(
                            out=pd[:, pcol:pcol + 1],
                            lhsT=tile[:, coff + c * CALL_IDX + b * P:
                                      coff + c * CALL_IDX + (b + 1) * P],
                            rhs=u_cols[:, c * GSEG + sloc:
                                       c * GSEG + sloc + 1],
                            start=False, stop=(c == 1),
                        )

            def emit_pd_copy(g):
                nc.vector.tensor_copy(
                    logits_sb[:, g * GRP_W:(g + 1) * GRP_W], pd_l[g][:])

            # ---- prologue ----
            emit_pos_group(0)
            emit_pos_group(1)
            # consts ride on the ACT lane ahead of its neg loads
            nc.scalar.dma_start(w_iT_sb[:], w_iT[:, :, :])
            nc.scalar.dma_start(wbT_sb[:], wbT[:, :, :])
            nc.scalar.dma_start(w_ext_sb[:], w_ext[:, :, :])
            nc.scalar.dma_start(b_iR_sb[:], b_iR[:, :])
            nc.scalar.dma_start(b_k_sb[:], b_k[:, :])
            emit_prep(0)

            # ---- main loop, prep pipelined one group ahead ----
            for g in range(NGRP):
                emit_cseed(g)
                # pos dots of group g
                for cal in range(2):
                    emit_dots(g, pos_tiles[2 * g + cal], 0,
                              [(cal * GB + b, b, cal * GB + b)
                               for b in range(GB)])
                for i in range(NEG_PER_GRP):
                    gi = g * NEG_PER_GRP + i
                    emit_neg(gi)
                    blocks = []
                    for b in range(GB):
                        ql = i * GB + b             # 0..79 within group
                        sl, r = ql // NEG_RATIO, ql % NEG_RATIO
                        blocks.append((GSEG + 16 * r + sl, b, sl))
                    emit_dots(g, neg_tiles[gi], 0, blocks)
                    if i == 2 and g + 1 < NGRP:
                        emit_prep(g + 1)
                    if i == 5 and g + 2 < NGRP:
                        emit_pos_group(g + 2)
                    if i == 7 and g > 0:
                        emit_pd_copy(g - 1)

            # store groups 0..6 while group 7's last loads are still in
            # flight; the final store is only 96 columns (cheap tail).
            nc.sync.dma_start(logits_d[:, :(NGRP - 1) * GRP_W],
                              logits_sb[:, :(NGRP - 1) * GRP_W])
            emit_pd_copy(NGRP - 1)
            nc.sync.dma_start(logits_d[:, (NGRP - 1) * GRP_W:],
                              logits_sb[:, (NGRP - 1) * GRP_W:])

    nc.compile()
    return nc


def get_module() -> bass.Bass:
    global _CACHED
    if _CACHED is None:
        _CACHED = _build_module()
    return _CACHED


def make_in_maps(inputs: dict) -> list[dict]:
    emb = np.ascontiguousarray(np.asarray(inputs["embedding"], dtype=np.float32))
    gs = np.asarray(inputs["grid_sizes"]).astype(np.int64)
    pos_s = np.asarray(inputs["pos_samples"]).astype(np.int64)
    neg_s = np.asarray(inputs["neg_samples"]).astype(np.int64)
    W_i = np.asarray(inputs["W_i"], dtype=np.float32)
    b_i = np.asarray(inputs["b_i"], dtype=np.float32)
    Wb = np.asarray(inputs["W_k"], dtype=np.float32)[0]
    b_kv = np.asarray(inputs["b_k"], dtype=np.float32)

    if not (gs.shape == (N_SEG,) and np.all(gs == SEG_SZ)):
        raise RuntimeError("kernel assumes grid_sizes == 128 everywhere")
    assert pos_s.shape == (N_POS,) and neg_s.shape == (N_NEG,)

    emb_bf = emb.astype(ml_dtypes.bfloat16)

    # mean = sum/seg_sz folded into the first chain matmul's weights
    w_iT_np = np.ascontiguousarray(
        (W_i / float(SEG_SZ)).T.reshape(2, P, H).transpose(1, 0, 2))
    wbT_np = np.ascontiguousarray(
        Wb.T.reshape(2, P, H).transpose(1, 0, 2))
    W_ext = np.concatenate([W_i, b_i[:, None]], axis=1)        # [256, 257]
    w_ext_np = np.ascontiguousarray(
        W_ext.reshape(2, P, H + 1).transpose(1, 0, 2))
    b_iR_np = b_i.reshape(1, H)
    b_k_np = b_kv.reshape(1, 1)

    in_maps = []
    for k in range(N_CORES):
        # processing order: pos rows then neg rows of this core, staged in
        # the device's transposed block layout:
        # rows[p, ci*2048 + c*1024 + n] = emb[full[ci*1024+n], c*128+p]
        full = np.concatenate([
            pos_s[k * POS_PC:(k + 1) * POS_PC],
            neg_s[k * NEG_PC:(k + 1) * NEG_PC],
        ])
        g = emb_bf[full]                       # [98304, 256]
        rows_np = np.ascontiguousarray(
            g.reshape(N_CALLS, CALL_IDX, 2, P).transpose(3, 0, 2, 1)
            .reshape(P, N_CALLS * CALL_W))
        in_maps.append({
            "rows": rows_np,
            "w_iT": w_iT_np,
            "wbT": wbT_np,
            "w_ext": w_ext_np,
            "b_iR": b_iR_np,
            "b_k": b_k_np,
        })
    return in_maps


def assemble_output(core_outs: list[np.ndarray]) -> np.ndarray:
    pos_parts, neg_parts = [], []
    for k in range(N_CORES):
        o = np.asarray(core_outs[k])
        assert o.shape == (P, TOT_BLK)
        o3 = o.reshape(P, NGRP, GRP_W)
        # pos: col 16*g + sl -> block 16*g + sl -> rows [block, p]
        pos_parts.append(np.ascontiguousarray(
            o3[:, :, :GSEG].transpose(1, 2, 0)).ravel())
        # neg: col 16 + 16*r + sl of group g -> q = 80*g + 5*sl + r
        neg_parts.append(np.ascontiguousarray(
            o3[:, :, GSEG:].reshape(P, NGRP, NEG_RATIO, GSEG)
            .transpose(1, 3, 2, 0)).ravel())
    return np.concatenate(pos_parts + neg_parts).astype(np.float32)


def kernel(**inputs) -> np.ndarray:
    nc = get_module()
    in_maps = make_in_maps(inputs)
    res = bass_utils.run_bass_kernel_spmd(
        nc, in_maps, core_ids=list(range(N_CORES)))
    return assemble_output([r["logits"] for r in res.results])
